# revision 1
# baseline (speedup 1.0000x reference)
"""Multi-head causal attention (B=4, T=2048, C=1024, H=16, D=64) on 8 trn2 cores.

Sharding: tensor-parallel over heads within batch core-pairs.
  core c -> batch b = c//2, heads hoff..hoff+7 where hoff = (c%2)*8.
Each core:
  - projects Q^T/K^T (head-pairs packed to 128 partitions) and V (head-quads
    packed, stride-65 layout with a ones column folded in for free softmax sums)
  - causal attention per head in S^T = [j, i] orientation, exp without
    max-subtraction (scores are ~N(0, 0.25^2), safe), fp32r matmuls throughout
  - output projection to partial y^T [1024 c', 2048 t] (+ bo/2)
  - pairwise ReduceScatter (4 t-slabs) sums partner partials; core even keeps
    c' 0:512, odd keeps c' 512:1024.
Host reassembles the [B, T, C] output by transposing/concatenating slabs.
"""

import numpy as np

import concourse.bass as bass
import concourse.mybir as mybir
from concourse import bacc
from concourse.tile import TileContext
from concourse.bass_utils import run_bass_kernel_spmd

F32 = mybir.dt.float32
F32R = mybir.dt.float32r

B, T, C = 4, 2048, 1024
H, D = 16, 64
HC = 8           # heads per core
NPAIR = HC // 2  # head pairs (QK packing)
CCn = C // 128   # 8 contraction chunks
TTn = T // 512   # 4 query tiles of 512
JCn = T // 128   # 16 key chunks of 128
N_CORES = 8
RG = [[0, 1], [2, 3], [4, 5], [6, 7]]


def build_nc(with_rs: bool = True):
    nc = bacc.Bacc(None, target_bir_lowering=False)

    xT = nc.declare_dram_parameter("xT", [C, T], F32R, isOutput=False)
    wq = nc.declare_dram_parameter("wq", [C, 512], F32R, isOutput=False)
    wk = nc.declare_dram_parameter("wk", [C, 512], F32R, isOutput=False)
    wv = nc.declare_dram_parameter("wv", [C, 512], F32R, isOutput=False)
    wot = nc.declare_dram_parameter("wot", [512, C], F32R, isOutput=False)
    bo2 = nc.declare_dram_parameter("bo2", [128, 8], F32, isOutput=False)
    y = nc.declare_dram_parameter("y", [TTn, 512, 512], F32, isOutput=True)

    with TileContext(nc) as tc:
        with (
            tc.tile_pool(name="persist", bufs=1) as persist,
            tc.tile_pool(name="psum", bufs=1, space="PSUM") as psum,
            tc.tile_pool(name="dram", bufs=1, space="DRAM") as dram,
        ):
            # ---- persistent tiles ----
            qt = [persist.tile([128, T], F32R, tag=f"qt{p}", name=f"qt{p}")
                  for p in range(NPAIR)]
            kt = [persist.tile([128, T], F32R, tag=f"kt{p}", name=f"kt{p}")
                  for p in range(NPAIR)]
            # V chunks: 8 heads * 65 cols (64 d + ones col for free softmax sums)
            v = [persist.tile([128, 65 * HC], F32R, tag=f"v{j}", name=f"v{j}")
                 for j in range(JCn)]
            ones8 = persist.tile([128, HC], F32, tag="ones8")
            nc.vector.memset(ones8[:], 1.0)
            ones1f = persist.tile([1, 64], F32, tag="ones1f")
            nc.vector.memset(ones1f[:], 1.0)
            ones1 = persist.tile([1, 64], F32R, tag="ones1")
            nc.vector.tensor_copy(ones1[:], ones1f[:])
            bo_sb = persist.tile([128, 8], F32, tag="bo_sb")
            nc.sync.dma_start(out=bo_sb[:], in_=bo2[:])
            pt_pool = persist

            y_part = dram.tile([TTn, 1024, 512], F32)
            rs_out = dram.tile([TTn, 512, 512], F32)

            # ---- phase A: projections, streamed by t-slab ----
            with tc.tile_pool(name="xw", bufs=1) as xw:
                wqt = [xw.tile([128, 512], F32R, tag=f"wq{cc}", name=f"wq{cc}")
                       for cc in range(CCn)]
                wkt = [xw.tile([128, 512], F32R, tag=f"wk{cc}", name=f"wk{cc}")
                       for cc in range(CCn)]
                wvt = [xw.tile([128, 512], F32R, tag=f"wv{cc}", name=f"wv{cc}")
                       for cc in range(CCn)]
                def issue_xts(tt):
                    i0 = tt * 512
                    xts = [xw.tile([128, 512], F32R, tag=f"xt{cc}", bufs=2,
                                   name=f"xt{cc}_{tt}") for cc in range(CCn)]
                    for cc in range(CCn):
                        nc.sync.dma_start(
                            out=xts[cc][:], in_=xT[cc * 128:(cc + 1) * 128, i0:i0 + 512]
                        )
                    return xts

                for cc in range(CCn):
                    nc.sync.dma_start(out=wqt[cc][:], in_=wq[cc * 128:(cc + 1) * 128, :])
                xts0 = issue_xts(0)
                for cc in range(CCn):
                    nc.sync.dma_start(out=wkt[cc][:], in_=wk[cc * 128:(cc + 1) * 128, :])
                for cc in range(CCn):
                    nc.sync.dma_start(out=wvt[cc][:], in_=wv[cc * 128:(cc + 1) * 128, :])

                for tt in range(TTn):
                    i0 = tt * 512
                    xts = xts0 if tt == 0 else issue_xts(tt)
                    for wt, dst in ((wqt, qt), (wkt, kt)):
                        for p2 in range(NPAIR // 2):
                            ps = psum.tile([128, 1024], F32, tag="stps", bufs=2,
                                           name=f"aps{tt}{p2}")
                            for k in range(2):
                                p = 2 * p2 + k
                                for cc in range(CCn):
                                    nc.tensor.matmul(
                                        ps[:, k * 512:(k + 1) * 512],
                                        wt[cc][:, p * 128:(p + 1) * 128],
                                        xts[cc][:],
                                        start=(cc == 0), stop=(cc == CCn - 1),
                                        skip_group_check=True,
                                    )
                            for k in range(2):
                                nc.vector.tensor_copy(
                                    dst[2 * p2 + k][:, i0:i0 + 512],
                                    ps[:, k * 512:(k + 1) * 512],
                                )
                    for jc in range(4 * tt, 4 * tt + 4):
                        jl = jc * 128 - i0  # 0..383 within slab
                        ps = psum.tile([128, 512], F32, tag="ovps", bufs=2,
                                       name=f"vps{jc}")
                        for g in range(2):
                            for cc in range(CCn):
                                nc.tensor.matmul(
                                    ps[:, g * 256:(g + 1) * 256],
                                    xts[cc][:, jl:jl + 128],
                                    wvt[cc][:, g * 256:(g + 1) * 256],
                                    start=(cc == 0), stop=(cc == CCn - 1),
                                    skip_group_check=True,
                                )
                        dst_ap = v[jc][:].rearrange(
                            "p (h e) -> p h e", h=HC, e=65
                        )[:, :, 0:64]
                        nc.vector.tensor_copy(dst_ap, ps[:])
                        ones_ap = v[jc][:].rearrange(
                            "p (h e) -> p h e", h=HC, e=65
                        )[:, :, 64:65]
                        nc.vector.tensor_copy(ones_ap, ones8[:])

            # ---- phase B/C interleaved per tt ----
            with tc.tile_pool(name="bc_pool", bufs=1) as bcp:
                ot = [bcp.tile([128, T], F32R, tag=f"ot{p}", name=f"ot{p}")
                      for p in range(NPAIR)]
                wot_t = [bcp.tile([128, C], F32R, tag=f"wot{cl}", name=f"wot{cl}")
                         for cl in range(4)]
                for cl in range(4):
                    nc.sync.dma_start(
                        out=wot_t[cl][:], in_=wot[cl * 128:(cl + 1) * 128, :]
                    )

                held = None  # (ov, h, pt, kk, n_jc) AV group awaiting emission

                def emit_norm(pend):
                    nonlocal held
                    ov, p, e, i0 = pend
                    if held is not None and held[0] is ov:
                        emit_avs(held)
                        held = None
                    # rows 0:64 = unnormalized O^T, row 64 = softmax sum l
                    rl = bcp.tile([1, 512], F32, tag="rl", bufs=2)
                    nc.vector.reciprocal(rl[:], ov[64:65, :])
                    rlr = bcp.tile([1, 512], F32R, tag="rlr", bufs=2)
                    nc.vector.tensor_copy(rlr[:], rl[:])
                    bc = psum.tile([64, 512], F32, tag="yps", bufs=2)
                    nc.tensor.matmul(
                        bc[:], ones1[:], rlr[:], start=True, stop=True,
                        skip_group_check=True,
                    )
                    bc_sb = bcp.tile([64, 512], F32, tag="bc_sb", bufs=2)
                    nc.vector.tensor_copy(bc_sb[:], bc[:])
                    nc.vector.tensor_mul(
                        ot[p][e * 64:(e + 1) * 64, i0:i0 + 512],
                        ov[0:64, :], bc_sb[:],
                    )

                pending = None

                def emit_outproj_group(tt, cp):
                    i0 = tt * 512
                    yps = psum.tile([128, 512], F32, tag="yps", bufs=2,
                                    name=f"yps{tt}{cp}")
                    for cl in range(4):
                        nc.tensor.matmul(
                            yps[:],
                            wot_t[cl][:, cp * 128:(cp + 1) * 128],
                            ot[cl][:, i0:i0 + 512],
                            start=(cl == 0), stop=(cl == 3),
                            skip_group_check=True,
                        )
                    ysb = bcp.tile([128, 512], F32, tag="ysb", bufs=4)
                    nc.vector.tensor_scalar_add(ysb[:], yps[:], bo_sb[:, cp:cp + 1])
                    nc.sync.dma_start(
                        out=y_part[tt, cp * 128:(cp + 1) * 128, :], in_=ysb[:]
                    )

                def emit_rs(tt):
                    if with_rs:
                        nc.gpsimd.collective_compute(
                            "ReduceScatter",
                            mybir.AluOpType.add,
                            replica_groups=RG,
                            ins=[y_part[tt]],
                            outs=[rs_out[tt]],
                        )
                        nc.sync.dma_start(out=y[tt], in_=rs_out[tt])
                    else:
                        nc.sync.dma_start(out=y[tt], in_=y_part[tt, 0:512, :])

                def emit_avs(held):
                    ov, h, pt, kk, n_jc = held
                    for k in range(2):
                        jc, a = kk[k]
                        nc.tensor.matmul(
                            ov[:, a:512],
                            v[jc][:, h * 65:(h + 1) * 65],
                            pt[:, k * 512 + a:(k + 1) * 512],
                            start=(jc == 0), stop=(jc == n_jc - 1),
                            skip_group_check=True,
                        )

                for tt in range(TTn):
                    i0 = tt * 512
                    n_jc = 4 * (tt + 1)
                    for h in range(HC):
                        p, e = h // 2, h % 2
                        ov = psum.tile([65, 512], F32, tag="ovps", bufs=2,
                                      name=f"ov{tt}{h}")
                        for jc2 in range(n_jc // 2):
                            st = psum.tile([128, 1024], F32, tag="stps", bufs=2,
                                          name=f"st{tt}{h}{jc2}")
                            kk = []  # (jc, a) for the two chunks
                            for k in range(2):
                                jc = 2 * jc2 + k
                                kb = jc - 4 * tt  # band offset (>=0 within band)
                                a = min(kb * 128, 256) if kb >= 0 else 0
                                kk.append((jc, a))
                                nc.tensor.matmul(
                                    st[:, k * 512 + a:(k + 1) * 512],
                                    kt[p][e * 64:(e + 1) * 64,
                                          jc * 128:(jc + 1) * 128],
                                    qt[p][e * 64:(e + 1) * 64,
                                          i0 + a:i0 + 512],
                                    start=True, stop=True,
                                    skip_group_check=True,
                                )
                            # AV of the previously-held group (keeps PE fed
                            # while ACT works on this group's exp); crosses
                            # head boundaries so head h+1's QK never waits on
                            # head h's last exp chain.
                            if held is not None:
                                emit_avs(held)
                                held = None
                            if jc2 == 0 and tt >= 1 and 1 <= h <= 2:
                                # previous slab's outproj, two groups per head
                                # over the first heads so its RS fires early
                                # enough to overlap this slab's compute instead
                                # of stacking behind the next RS. norm(tt-1,h7)
                                # lands at (tt,h0,jc2==1), before these reads.
                                for g4 in range(4):
                                    emit_outproj_group(tt - 1, 4 * (h - 1) + g4)
                                if h == 2:
                                    emit_rs(tt - 1)
                            if jc2 == 1 and pending is not None:
                                emit_norm(pending)
                                pending = None
                            pt = pt_pool.tile([128, 1024], F32R, tag="pt", bufs=7,
                                              name=f"pt{tt}{h}{jc2}")
                            a0 = kk[0][1]
                            nc.scalar.activation(
                                pt[:, a0:1024], st[:, a0:1024],
                                mybir.ActivationFunctionType.Exp,
                            )
                            if kk[0][1] == 256:
                                # merged causal zeroing for band pair (k2,k3):
                                # slices [256:512],[768:1024]; iota = f - p - 128*o
                                sel = pt[:].rearrange(
                                    "p (o i) -> p o i", o=2, i=512)[:, :, 256:512]
                                nc.gpsimd.affine_select(
                                    out=sel, in_=sel,
                                    compare_op=mybir.AluOpType.is_ge,
                                    fill=0.0, base=0,
                                    pattern=[[-128, 2], [1, 256]],
                                    channel_multiplier=-1,
                                )
                            else:
                                for k in range(2):
                                    jc, a = kk[k]
                                    if jc >= 4 * tt:  # diag band chunk
                                        nc.gpsimd.affine_select(
                                            out=pt[:, k * 512 + a:(k + 1) * 512],
                                            in_=pt[:, k * 512 + a:(k + 1) * 512],
                                            compare_op=mybir.AluOpType.is_ge,
                                            fill=0.0,
                                            base=a - (jc - 4 * tt) * 128,
                                            pattern=[[1, 512 - a]],
                                            channel_multiplier=-1,
                                        )
                            held = (ov, h, pt, kk, n_jc)
                        if pending is not None:  # tt0 heads have only 2 groups
                            emit_norm(pending)
                        pending = (ov, p, e, i0)
                    if tt == TTn - 1:
                        if pending is not None:
                            emit_norm(pending)
                            pending = None
                        for cp in range(8):
                            emit_outproj_group(tt, cp)
                        emit_rs(tt)

    nc.compile()
    return nc


_NC_CACHE = {}


def _get_nc(with_rs: bool = True):
    key = bool(with_rs)
    if key not in _NC_CACHE:
        _NC_CACHE[key] = build_nc(with_rs)
    return _NC_CACHE[key]


def make_in_maps(x, Wq, Wk, Wv, Wo, bo):
    x = np.asarray(x, dtype=np.float32)
    Wq = np.asarray(Wq, dtype=np.float32)
    Wk = np.asarray(Wk, dtype=np.float32)
    Wv = np.asarray(Wv, dtype=np.float32)
    Wo = np.asarray(Wo, dtype=np.float32)
    bo = np.asarray(bo, dtype=np.float32)

    scale = np.float32(C) ** np.float32(-0.5)
    in_maps = []
    for c in range(N_CORES):
        b, hoff = c // 2, (c % 2) * HC
        heads = slice(hoff, hoff + HC)
        xT_c = np.ascontiguousarray(x[b].T)                      # [C, T]
        wq_c = np.ascontiguousarray(
            np.concatenate(list(Wq[heads] * scale), axis=1))     # [C, 512]
        wk_c = np.ascontiguousarray(np.concatenate(list(Wk[heads]), axis=1))
        wv_c = np.ascontiguousarray(np.concatenate(list(Wv[heads]), axis=1))
        wot_c = np.ascontiguousarray(Wo[:, hoff * D:(hoff + HC) * D].T)  # [512, C]
        bo2_c = np.ascontiguousarray((bo / 2.0).reshape(8, 128).T)       # [128, 8]
        in_maps.append({
            "xT": xT_c, "wq": wq_c, "wk": wk_c, "wv": wv_c,
            "wot": wot_c, "bo2": bo2_c,
        })
    return in_maps


def kernel(x, Wq, Wk, Wv, Wo, bo):
    nc = _get_nc(with_rs=True)
    in_maps = make_in_maps(x, Wq, Wk, Wv, Wo, bo)
    # The axon-tunneled devices occasionally fail transiently
    # (NRT_EXEC_UNIT_UNRECOVERABLE / tunnel hangup); a retry recovers.
    last_err = None
    for _ in range(3):
        try:
            res = run_bass_kernel_spmd(nc, in_maps, list(range(N_CORES))).results
            break
        except Exception as e:  # noqa: BLE001
            last_err = e
            import time
            time.sleep(5)
    else:
        raise last_err

    out = np.empty((B, T, C), dtype=np.float32)
    for c in range(N_CORES):
        b, e = c // 2, c % 2
        yc = res[c]["y"]  # [4, 512, 512] = [tt, c' slab, t]
        for tt in range(TTn):
            out[b, tt * 512:(tt + 1) * 512, e * 512:(e + 1) * 512] = yc[tt].T
    return out



# revision 7
# speedup vs baseline: 1.1997x; 1.1997x over previous
"""Multi-head causal attention (B=4, T=2048, C=1024, H=16, D=64) on 8 trn2 cores.

Sharding: tensor-parallel over heads within batch core-pairs.
  core c -> batch b = c//2, heads hoff..hoff+7 where hoff = (c%2)*8.

Per-core pipeline (all matmul operands bf16, accumulation f32 in PSUM):
  - projections per t-slab (Q^T/K^T head-pair packed to 128 partitions; V
    head-packed with a ones column folded in for free softmax sums);
    slab tt+1's projections are interleaved into slab tt's attention so the
    PE fills the exp-latency gaps (attention is ACT-bound).
  - causal attention per head in S^T = [j, i] orientation, exp without
    max-subtraction (scores ~N(0, 0.25^2), safe); merged affine_select
    causal masks (one per diagonal chunk-pair).
  - AV in flipped orientation: stationary = P^T chunk [128 k, 128 q],
    moving = V [128 k, 65] -> O accumulates as [q, d|l] in PSUM; fully
    masked (q-chunk < key-chunk) matmuls skipped. Softmax normalization is
    then a per-partition tensor_scalar multiply; O^T rebuilt with PE
    transposes for the output projection.
  - output projection to partial y^T [1024 c', 2048 t] (+ bo/2) in bf16
  - pairwise ReduceScatter (bf16, per t-slab) sums partner partials; core
    even keeps c' 0:512, odd keeps c' 512:1024.
Host reassembles the [B, T, C] f32 output by transposing/concatenating.
"""

import numpy as np

import concourse.bass as bass
import concourse.mybir as mybir
from concourse import bacc, masks
from concourse.tile import TileContext
from concourse.bass_utils import run_bass_kernel_spmd

F32 = mybir.dt.float32
BF16 = mybir.dt.bfloat16

B, T, C = 4, 2048, 1024
H, D = 16, 64
HC = 8           # heads per core
NPAIR = HC // 2  # head pairs (QK packing)
CCn = C // 128   # 8 contraction chunks
TTn = T // 512   # 4 query slabs of 512
JCn = T // 128   # 16 key chunks of 128
N_CORES = 8
RG = [[0, 1], [2, 3], [4, 5], [6, 7]]


def build_nc(with_rs: bool = True):
    nc = bacc.Bacc(None, target_bir_lowering=False)

    xT = nc.declare_dram_parameter("xT", [C, T], BF16, isOutput=False)
    wq = nc.declare_dram_parameter("wq", [C, 512], BF16, isOutput=False)
    wk = nc.declare_dram_parameter("wk", [C, 512], BF16, isOutput=False)
    wv = nc.declare_dram_parameter("wv", [C, 512], BF16, isOutput=False)
    wot = nc.declare_dram_parameter("wot", [512, C], BF16, isOutput=False)
    bo2 = nc.declare_dram_parameter("bo2", [128, 8], F32, isOutput=False)
    y = nc.declare_dram_parameter("y", [TTn, 512, 512], BF16, isOutput=True)

    with TileContext(nc) as tc:
        with (
            tc.tile_pool(name="persist", bufs=1) as pp,
            tc.tile_pool(name="psum", bufs=1, space="PSUM") as psum,
            tc.tile_pool(name="dram", bufs=1, space="DRAM") as dram,
        ):
            # ---- persistent tiles ----
            qt = [pp.tile([128, T], BF16, tag=f"qt{p}", name=f"qt{p}")
                  for p in range(NPAIR)]
            kt = [pp.tile([128, T], BF16, tag=f"kt{p}", name=f"kt{p}")
                  for p in range(NPAIR)]
            # V chunks: 8 heads * 65 cols (64 d + ones col for softmax sums)
            v = [pp.tile([128, 65 * HC], BF16, tag=f"v{j}", name=f"v{j}")
                 for j in range(JCn)]
            ot = [pp.tile([128, T], BF16, tag=f"ot{p}", name=f"ot{p}")
                  for p in range(NPAIR)]
            wqt = [pp.tile([128, 512], BF16, tag=f"wq{cc}", name=f"wq{cc}")
                   for cc in range(CCn)]
            wkt = [pp.tile([128, 512], BF16, tag=f"wk{cc}", name=f"wk{cc}")
                   for cc in range(CCn)]
            wvt = [pp.tile([128, 512], BF16, tag=f"wv{cc}", name=f"wv{cc}")
                   for cc in range(CCn)]
            wot_t = [pp.tile([128, C], BF16, tag=f"wot{cl}", name=f"wot{cl}")
                     for cl in range(4)]
            ones8 = pp.tile([128, HC], BF16, tag="ones8")
            nc.vector.memset(ones8[:], 1.0)
            ident = pp.tile([128, 128], BF16, tag="ident")
            masks.make_identity(nc, ident[:])
            bo_sb = pp.tile([128, 8], F32, tag="bo_sb")

            y_part = dram.tile([TTn, 1024, 512], BF16)
            rs_out = dram.tile([TTn, 512, 512], BF16)

            # ---- startup DMAs: wq/x0 interleaved so Q proj starts early ----
            xts_store = {}

            def issue_xts(tt, interleave_with=None):
                i0 = tt * 512
                xts = [pp.tile([128, 512], BF16, tag=f"xt{cc}", bufs=2,
                               name=f"xt{cc}_{tt}") for cc in range(CCn)]
                for cc in range(CCn):
                    if interleave_with is not None:
                        nc.sync.dma_start(out=interleave_with[cc][:],
                                          in_=wq[cc * 128:(cc + 1) * 128, :])
                    nc.sync.dma_start(
                        out=xts[cc][:], in_=xT[cc * 128:(cc + 1) * 128, i0:i0 + 512]
                    )
                xts_store[tt] = xts

            issue_xts(0, interleave_with=wqt)
            for cc in range(CCn):
                nc.sync.dma_start(out=wkt[cc][:], in_=wk[cc * 128:(cc + 1) * 128, :])
            for cc in range(CCn):
                nc.sync.dma_start(out=wvt[cc][:], in_=wv[cc * 128:(cc + 1) * 128, :])
            for cl in range(4):
                nc.sync.dma_start(out=wot_t[cl][:],
                                  in_=wot[cl * 128:(cl + 1) * 128, :])
            nc.sync.dma_start(out=bo_sb[:], in_=bo2[:])

            # ---- projection emission (phase A), one group at a time ----
            def emit_a_group(tt, gi):
                """gi 0-1: Q pair-halves; 2-3: K; 4-7: V chunks."""
                i0 = tt * 512
                xts = xts_store[tt]
                if gi < 4:
                    wt, dst = (wqt, qt) if gi < 2 else (wkt, kt)
                    p2 = gi % 2
                    ps = psum.tile([128, 1024], F32, tag="st", bufs=2,
                                   name=f"aps{tt}_{gi}")
                    for k in range(2):
                        p = 2 * p2 + k
                        for cc in range(CCn):
                            nc.tensor.matmul(
                                ps[:, k * 512:(k + 1) * 512],
                                wt[cc][:, p * 128:(p + 1) * 128],
                                xts[cc][:],
                                start=(cc == 0), stop=(cc == CCn - 1),
                                skip_group_check=True,
                            )
                    for k in range(2):
                        nc.vector.tensor_copy(
                            dst[2 * p2 + k][:, i0:i0 + 512],
                            ps[:, k * 512:(k + 1) * 512],
                        )
                else:
                    jc = 4 * tt + (gi - 4)
                    jl = jc * 128 - i0
                    ps = psum.tile([128, 512], F32, tag="yps", bufs=2,
                                   name=f"vps{jc}")
                    for g in range(2):
                        for cc in range(CCn):
                            nc.tensor.matmul(
                                ps[:, g * 256:(g + 1) * 256],
                                xts[cc][:, jl:jl + 128],
                                wvt[cc][:, g * 256:(g + 1) * 256],
                                start=(cc == 0), stop=(cc == CCn - 1),
                                skip_group_check=True,
                            )
                    vre = v[jc][:].rearrange("p (h e) -> p h e", h=HC, e=65)
                    nc.vector.tensor_copy(vre[:, :, 0:64], ps[:])
                    nc.vector.tensor_copy(vre[:, :, 64:65], ones8[:])

            # ---- attention phase ----
            held = None     # AV batch awaiting emission
            pending = None  # (ov, h, tt) awaiting normalization

            def emit_avs(h_):
                # whole-head AV burst, qc-major: PSUM allows only one open
                # accumulation group per bank, so each qc region's chunk
                # accumulation must be contiguous.
                ov, h, tt, pts = h_
                for qc in range(4):
                    for jc in range(4 * tt + qc + 1):
                        g2, k = jc // 2, jc % 2
                        nc.tensor.matmul(
                            ov[:, qc * 65:(qc + 1) * 65],
                            pts[g2][:, k * 512 + qc * 128:k * 512 + (qc + 1) * 128],
                            v[jc][:, h * 65:(h + 1) * 65],
                            start=(jc == 0), stop=(jc == 4 * tt + qc),
                            skip_group_check=True,
                        )

            def emit_norm(pend):
                nonlocal held
                ov, h, tt = pend
                if held is not None and held[0] is ov:
                    emit_avs(held)
                    held = None
                p, e, i0 = h // 2, h % 2, tt * 512
                ovv = ov[:].rearrange("p (q f) -> p q f", q=4, f=65)
                rl4 = pp.tile([128, 4], F32, tag="rl4", bufs=2)
                nc.vector.reciprocal(rl4[:], ovv[:, :, 64:65])
                o_sb = pp.tile([128, 256], BF16, tag="osb", bufs=2)
                for qc in range(4):
                    nc.vector.tensor_scalar_mul(
                        o_sb[:, qc * 64:(qc + 1) * 64],
                        ov[:, qc * 65:qc * 65 + 64],
                        rl4[:, qc:qc + 1],
                    )
                tps = psum.tile([64, 512], BF16, tag="yps", bufs=2,
                                name=f"tps{tt}{h}")
                for qc in range(4):
                    nc.tensor.matmul(
                        tps[:, qc * 128:(qc + 1) * 128],
                        o_sb[:, qc * 64:(qc + 1) * 64],
                        ident[:],
                        is_transpose=True, start=True, stop=True,
                        skip_group_check=True,
                    )
                nc.vector.tensor_copy(ot[p][e * 64:(e + 1) * 64, i0:i0 + 512],
                                      tps[:])

            def emit_outproj_group(tt, cp):
                i0 = tt * 512
                yps = psum.tile([128, 512], F32, tag="yps", bufs=2,
                                name=f"yps{tt}{cp}")
                for cl in range(4):
                    nc.tensor.matmul(
                        yps[:],
                        wot_t[cl][:, cp * 128:(cp + 1) * 128],
                        ot[cl][:, i0:i0 + 512],
                        start=(cl == 0), stop=(cl == 3),
                        skip_group_check=True,
                    )
                ysb = pp.tile([128, 512], BF16, tag="ysb", bufs=4)
                nc.vector.tensor_scalar_add(ysb[:], yps[:], bo_sb[:, cp:cp + 1])
                nc.sync.dma_start(
                    out=y_part[tt, cp * 128:(cp + 1) * 128, :], in_=ysb[:]
                )

            def emit_rs(tt):
                if with_rs:
                    nc.gpsimd.collective_compute(
                        "ReduceScatter",
                        mybir.AluOpType.add,
                        replica_groups=RG,
                        ins=[y_part[tt]],
                        outs=[rs_out[tt]],
                    )
                    nc.sync.dma_start(out=y[tt], in_=rs_out[tt])
                else:
                    nc.sync.dma_start(out=y[tt], in_=y_part[tt, 0:512, :])

            # standalone projections for slab 0
            for gi in range(8):
                emit_a_group(0, gi)

            for tt in range(TTn):
                i0 = tt * 512
                n_jc = 4 * (tt + 1)
                ngroups = n_jc // 2
                if tt < TTn - 1:
                    issue_xts(tt + 1)
                for h in range(HC):
                    p, e = h // 2, h % 2
                    ov = psum.tile([128, 260], F32, tag="ov", bufs=2,
                                   name=f"ov{tt}{h}")
                    pts = []
                    for g in range(ngroups):
                        st = psum.tile([128, 1024], F32, tag="st", bufs=2,
                                       name=f"st{tt}{h}{g}")
                        kk = []
                        for k in range(2):
                            jc = 2 * g + k
                            kb = jc - 4 * tt
                            a = min(kb * 128, 256) if kb >= 0 else 0
                            kk.append((jc, a))
                            nc.tensor.matmul(
                                st[:, k * 512 + a:(k + 1) * 512],
                                kt[p][e * 64:(e + 1) * 64,
                                      jc * 128:(jc + 1) * 128],
                                qt[p][e * 64:(e + 1) * 64, i0 + a:i0 + 512],
                                start=True, stop=True,
                                skip_group_check=True,
                            )
                        # AV of the previously-held group keeps PE fed while
                        # ACT runs this group's exp; crosses head boundaries.
                        if held is not None:
                            emit_avs(held)
                            held = None
                        if g == 0 and tt >= 1 and 1 <= h <= 2:
                            # previous slab's outproj + RS, early enough to
                            # overlap this slab's compute
                            for g4 in range(4):
                                emit_outproj_group(tt - 1, 4 * (h - 1) + g4)
                            if h == 2:
                                emit_rs(tt - 1)
                        if g == 1 and pending is not None:
                            emit_norm(pending)
                            pending = None
                        if g == ngroups - 1 and tt < TTn - 1:
                            # next slab's projections as PE filler
                            emit_a_group(tt + 1, h)
                        pt = pp.tile([128, 1024], BF16, tag="pt", bufs=10,
                                     name=f"pt{tt}{h}{g}")
                        a0 = kk[0][1]
                        nc.scalar.activation(
                            pt[:, a0:1024], st[:, a0:1024],
                            mybir.ActivationFunctionType.Exp,
                        )
                        kb0 = 2 * g - 4 * tt
                        if kb0 >= 0:
                            # merged causal zeroing for diag pair (kb0, kb0+1):
                            # iota = i - 128*o - p over block-local cols
                            off = kb0 * 128
                            sel = pt[:].rearrange(
                                "p (o i) -> p o i", o=2, i=512
                            )[:, :, off:off + 256]
                            nc.gpsimd.affine_select(
                                out=sel, in_=sel,
                                compare_op=mybir.AluOpType.is_ge,
                                fill=0.0, base=0,
                                pattern=[[-128, 2], [1, 256]],
                                channel_multiplier=-1,
                            )
                        pts.append(pt)
                    held = (ov, h, tt, pts)
                    if pending is not None:
                        emit_norm(pending)
                        pending = None
                    pending = (ov, h, tt)
                if tt == TTn - 1:
                    if pending is not None:
                        emit_norm(pending)
                        pending = None
                    for cp in range(8):
                        emit_outproj_group(tt, cp)
                    emit_rs(tt)

    nc.compile()
    return nc


_NC_CACHE = {}


def _get_nc(with_rs: bool = True):
    key = bool(with_rs)
    if key not in _NC_CACHE:
        _NC_CACHE[key] = build_nc(with_rs)
    return _NC_CACHE[key]


def make_in_maps(x, Wq, Wk, Wv, Wo, bo):
    import ml_dtypes
    bf16 = ml_dtypes.bfloat16

    x = np.asarray(x, dtype=np.float32)
    Wq = np.asarray(Wq, dtype=np.float32)
    Wk = np.asarray(Wk, dtype=np.float32)
    Wv = np.asarray(Wv, dtype=np.float32)
    Wo = np.asarray(Wo, dtype=np.float32)
    bo = np.asarray(bo, dtype=np.float32)

    scale = np.float32(C) ** np.float32(-0.5)
    in_maps = []
    for c in range(N_CORES):
        b, hoff = c // 2, (c % 2) * HC
        heads = slice(hoff, hoff + HC)
        xT_c = np.ascontiguousarray(x[b].T.astype(bf16))              # [C, T]
        wq_c = np.ascontiguousarray(
            np.concatenate(list(Wq[heads] * scale), axis=1).astype(bf16))
        wk_c = np.ascontiguousarray(
            np.concatenate(list(Wk[heads]), axis=1).astype(bf16))
        wv_c = np.ascontiguousarray(
            np.concatenate(list(Wv[heads]), axis=1).astype(bf16))
        wot_c = np.ascontiguousarray(
            Wo[:, hoff * D:(hoff + HC) * D].T.astype(bf16))           # [512, C]
        bo2_c = np.ascontiguousarray((bo / 2.0).reshape(8, 128).T)    # [128, 8]
        in_maps.append({
            "xT": xT_c, "wq": wq_c, "wk": wk_c, "wv": wv_c,
            "wot": wot_c, "bo2": bo2_c,
        })
    return in_maps


def kernel(x, Wq, Wk, Wv, Wo, bo):
    nc = _get_nc(with_rs=True)
    in_maps = make_in_maps(x, Wq, Wk, Wv, Wo, bo)
    # The axon-tunneled devices occasionally fail transiently
    # (NRT_EXEC_UNIT_UNRECOVERABLE / tunnel hangup); a retry recovers.
    last_err = None
    for _ in range(3):
        try:
            res = run_bass_kernel_spmd(nc, in_maps, list(range(N_CORES))).results
            break
        except Exception as e:  # noqa: BLE001
            last_err = e
            import time
            time.sleep(5)
    else:
        raise last_err

    out = np.empty((B, T, C), dtype=np.float32)
    for c in range(N_CORES):
        b, e = c // 2, c % 2
        yc = np.asarray(res[c]["y"], dtype=np.float32)  # [tt, c' slab, t]
        for tt in range(TTn):
            out[b, tt * 512:(tt + 1) * 512, e * 512:(e + 1) * 512] = yc[tt].T
    return out


# revision 12
# speedup vs baseline: 1.2360x; 1.0303x over previous
"""Multi-head causal attention (B=4, T=2048, C=1024, H=16, D=64) on 8 trn2 cores.

Sharding: tensor-parallel over heads within batch core-pairs.
  core c -> batch b = c//2, heads hoff..hoff+7 where hoff = (c%2)*8.

Per-core pipeline (all matmul operands bf16, accumulation f32 in PSUM):
  - projections per t-slab (Q^T/K^T head-pair packed to 128 partitions; V
    head-packed with a ones column folded in for free softmax sums);
    slab tt+1's projections are interleaved into slab tt's attention so the
    PE fills the exp-latency gaps (attention is ACT-bound).
  - causal attention per head in S^T = [j, i] orientation, exp without
    max-subtraction (scores ~N(0, 0.25^2), safe); merged affine_select
    causal masks (one per diagonal chunk-pair).
  - AV in flipped orientation: stationary = P^T chunk [128 k, 128 q],
    moving = V [128 k, 65] -> O accumulates as [q, d|l] in PSUM; fully
    masked (q-chunk < key-chunk) matmuls skipped. Softmax normalization is
    then a per-partition tensor_scalar multiply; O^T rebuilt with PE
    transposes for the output projection.
  - output projection to partial y^T [1024 c', 2048 t] (+ bo/2) in bf16
  - pairwise ReduceScatter (bf16, per t-slab) sums partner partials; core
    even keeps c' 0:512, odd keeps c' 512:1024.
Host reassembles the [B, T, C] f32 output by transposing/concatenating.
"""

import numpy as np

import concourse.bass as bass
import concourse.mybir as mybir
from concourse import bacc, masks
from concourse.tile import TileContext
from concourse.bass_utils import run_bass_kernel_spmd

F32 = mybir.dt.float32
BF16 = mybir.dt.bfloat16

B, T, C = 4, 2048, 1024
H, D = 16, 64
HC = 8           # heads per core
NPAIR = HC // 2  # head pairs (QK packing)
CCn = C // 128   # 8 contraction chunks
TTn = T // 512   # 4 query slabs of 512
JCn = T // 128   # 16 key chunks of 128
N_CORES = 8
RG = [[0, 1], [2, 3], [4, 5], [6, 7]]


def build_nc(with_rs: bool = True):
    nc = bacc.Bacc(None, target_bir_lowering=False)

    xT = nc.declare_dram_parameter("xT", [C, T], BF16, isOutput=False)
    wq = nc.declare_dram_parameter("wq", [C, 512], BF16, isOutput=False)
    wk = nc.declare_dram_parameter("wk", [C, 512], BF16, isOutput=False)
    wv = nc.declare_dram_parameter("wv", [C, 512], BF16, isOutput=False)
    wot = nc.declare_dram_parameter("wot", [512, C], BF16, isOutput=False)
    bo2 = nc.declare_dram_parameter("bo2", [128, 8], F32, isOutput=False)
    y = nc.declare_dram_parameter("y", [TTn, 512, 512], BF16, isOutput=True)

    with TileContext(nc) as tc:
        with (
            tc.tile_pool(name="persist", bufs=1) as pp,
            tc.tile_pool(name="psum", bufs=1, space="PSUM") as psum,
            tc.tile_pool(name="dram", bufs=1, space="DRAM") as dram,
        ):
            # ---- persistent tiles ----
            qt = [pp.tile([128, T], BF16, tag=f"qt{p}", name=f"qt{p}")
                  for p in range(NPAIR)]
            kt = [pp.tile([128, T], BF16, tag=f"kt{p}", name=f"kt{p}")
                  for p in range(NPAIR)]
            # V chunks: 8 heads * 65 cols (64 d + ones col for softmax sums)
            v = [pp.tile([128, 65 * HC], BF16, tag=f"v{j}", name=f"v{j}")
                 for j in range(JCn)]
            ot = [pp.tile([128, T], BF16, tag=f"ot{p}", name=f"ot{p}")
                  for p in range(NPAIR)]
            wqt = [pp.tile([128, 512], BF16, tag=f"wq{cc}", name=f"wq{cc}")
                   for cc in range(CCn)]
            wkt = [pp.tile([128, 512], BF16, tag=f"wk{cc}", name=f"wk{cc}")
                   for cc in range(CCn)]
            wvt = [pp.tile([128, 512], BF16, tag=f"wv{cc}", name=f"wv{cc}")
                   for cc in range(CCn)]
            wot_t = [pp.tile([128, C], BF16, tag=f"wot{cl}", name=f"wot{cl}")
                     for cl in range(4)]
            ones8 = pp.tile([128, HC], BF16, tag="ones8")
            nc.vector.memset(ones8[:], 1.0)
            ident = pp.tile([128, 128], BF16, tag="ident")
            masks.make_identity(nc, ident[:])
            bo_sb = pp.tile([128, 8], F32, tag="bo_sb")

            y_part = dram.tile([TTn, 1024, 512], BF16)
            rs_out = dram.tile([TTn, 512, 512], BF16)

            # ---- startup DMAs: wq/x0 interleaved so Q proj starts early ----
            xts_store = {}

            def issue_xts(tt, interleave_with=None):
                i0 = tt * 512
                xts = [pp.tile([128, 512], BF16, tag=f"xt{cc}", bufs=2,
                               name=f"xt{cc}_{tt}") for cc in range(CCn)]
                for cc in range(CCn):
                    if interleave_with is not None:
                        nc.sync.dma_start(out=interleave_with[cc][:],
                                          in_=wq[cc * 128:(cc + 1) * 128, :])
                    nc.sync.dma_start(
                        out=xts[cc][:], in_=xT[cc * 128:(cc + 1) * 128, i0:i0 + 512]
                    )
                xts_store[tt] = xts

            issue_xts(0, interleave_with=wqt)
            for cc in range(CCn):
                nc.sync.dma_start(out=wkt[cc][:], in_=wk[cc * 128:(cc + 1) * 128, :])
            for cc in range(CCn):
                nc.sync.dma_start(out=wvt[cc][:], in_=wv[cc * 128:(cc + 1) * 128, :])
            for cl in range(4):
                nc.sync.dma_start(out=wot_t[cl][:],
                                  in_=wot[cl * 128:(cl + 1) * 128, :])
            nc.sync.dma_start(out=bo_sb[:], in_=bo2[:])

            # ---- projection emission (phase A), one group at a time ----
            def emit_a_group(tt, gi):
                """gi 0-1: Q pair-halves; 2-3: K; 4-7: V chunks."""
                i0 = tt * 512
                xts = xts_store[tt]
                if gi < 4:
                    # yps tag (not st): keeps this filler work off the QK
                    # score-psum rotation so it can't stall the exp pipeline
                    wt, dst = (wqt, qt) if gi < 2 else (wkt, kt)
                    p2 = gi % 2
                    for k in range(2):
                        p = 2 * p2 + k
                        ps = psum.tile([128, 512], F32, tag="yps", bufs=2,
                                       name=f"aps{tt}_{gi}_{k}")
                        for cc in range(CCn):
                            nc.tensor.matmul(
                                ps[:],
                                wt[cc][:, p * 128:(p + 1) * 128],
                                xts[cc][:],
                                start=(cc == 0), stop=(cc == CCn - 1),
                                skip_group_check=True,
                            )
                        nc.vector.tensor_copy(dst[p][:, i0:i0 + 512], ps[:])
                else:
                    jc = 4 * tt + (gi - 4)
                    jl = jc * 128 - i0
                    ps = psum.tile([128, 512], F32, tag="yps", bufs=2,
                                   name=f"vps{jc}")
                    for g in range(2):
                        for cc in range(CCn):
                            nc.tensor.matmul(
                                ps[:, g * 256:(g + 1) * 256],
                                xts[cc][:, jl:jl + 128],
                                wvt[cc][:, g * 256:(g + 1) * 256],
                                start=(cc == 0), stop=(cc == CCn - 1),
                                skip_group_check=True,
                            )
                    vre = v[jc][:].rearrange("p (h e) -> p h e", h=HC, e=65)
                    nc.vector.tensor_copy(vre[:, :, 0:64], ps[:])
                    nc.vector.tensor_copy(vre[:, :, 64:65], ones8[:])

            # ---- attention phase ----
            held = None     # AV batch awaiting emission
            pending = None  # (ov, h, tt) awaiting normalization

            def emit_avs(h_):
                # whole-head AV burst, qc-major: PSUM allows only one open
                # accumulation group per bank, so each qc region's chunk
                # accumulation must be contiguous.
                ov, h, tt, pts = h_
                for qc in range(4):
                    for jc in range(4 * tt + qc + 1):
                        g2, k = jc // 2, jc % 2
                        nc.tensor.matmul(
                            ov[:, qc * 65:(qc + 1) * 65],
                            pts[g2][:, k * 512 + qc * 128:k * 512 + (qc + 1) * 128],
                            v[jc][:, h * 65:(h + 1) * 65],
                            start=(jc == 0), stop=(jc == 4 * tt + qc),
                            skip_group_check=True,
                        )

            def emit_norm(pend):
                nonlocal held
                ov, h, tt = pend
                if held is not None and held[0] is ov:
                    emit_avs(held)
                    held = None
                p, e, i0 = h // 2, h % 2, tt * 512
                ovv = ov[:].rearrange("p (q f) -> p q f", q=4, f=65)
                rl4 = pp.tile([128, 4], F32, tag="rl4", bufs=2)
                nc.vector.reciprocal(rl4[:], ovv[:, :, 64:65])
                o_sb = pp.tile([128, 256], BF16, tag="osb", bufs=2)
                for qc in range(4):
                    nc.vector.tensor_scalar_mul(
                        o_sb[:, qc * 64:(qc + 1) * 64],
                        ov[:, qc * 65:qc * 65 + 64],
                        rl4[:, qc:qc + 1],
                    )
                tps = psum.tile([64, 512], BF16, tag="yps", bufs=2,
                                name=f"tps{tt}{h}")
                for qc in range(4):
                    nc.tensor.matmul(
                        tps[:, qc * 128:(qc + 1) * 128],
                        o_sb[:, qc * 64:(qc + 1) * 64],
                        ident[:],
                        is_transpose=True, start=True, stop=True,
                        skip_group_check=True,
                    )
                nc.vector.tensor_copy(ot[p][e * 64:(e + 1) * 64, i0:i0 + 512],
                                      tps[:])

            def emit_outproj_group(tt, cp):
                i0 = tt * 512
                yps = psum.tile([128, 512], F32, tag="yps", bufs=2,
                                name=f"yps{tt}{cp}")
                for cl in range(4):
                    nc.tensor.matmul(
                        yps[:],
                        wot_t[cl][:, cp * 128:(cp + 1) * 128],
                        ot[cl][:, i0:i0 + 512],
                        start=(cl == 0), stop=(cl == 3),
                        skip_group_check=True,
                    )
                ysb = pp.tile([128, 512], BF16, tag="ysb", bufs=4)
                nc.vector.tensor_scalar_add(ysb[:], yps[:], bo_sb[:, cp:cp + 1])
                nc.sync.dma_start(
                    out=y_part[tt, cp * 128:(cp + 1) * 128, :], in_=ysb[:]
                )

            def emit_rs(tt):
                if with_rs:
                    nc.gpsimd.collective_compute(
                        "ReduceScatter",
                        mybir.AluOpType.add,
                        replica_groups=RG,
                        ins=[y_part[tt]],
                        outs=[rs_out[tt]],
                    )
                    nc.sync.dma_start(out=y[tt], in_=rs_out[tt])
                else:
                    nc.sync.dma_start(out=y[tt], in_=y_part[tt, 0:512, :])

            # standalone projections for slab 0
            for gi in range(8):
                emit_a_group(0, gi)

            for tt in range(TTn):
                i0 = tt * 512
                n_jc = 4 * (tt + 1)
                ngroups = n_jc // 2
                if tt < TTn - 1:
                    issue_xts(tt + 1)
                for h in range(HC):
                    p, e = h // 2, h % 2
                    ov = psum.tile([128, 260], F32, tag="ov", bufs=2,
                                   name=f"ov{tt}{h}")
                    pts = []
                    for g in range(ngroups):
                        st = psum.tile([128, 1024], F32, tag="st", bufs=2,
                                       name=f"st{tt}{h}{g}")
                        kk = []
                        for k in range(2):
                            jc = 2 * g + k
                            kb = jc - 4 * tt
                            cap = 256 if k == 0 else 384
                            a = min(kb * 128, cap) if kb >= 0 else 0
                            kk.append((jc, a))
                            nc.tensor.matmul(
                                st[:, k * 512 + a:(k + 1) * 512],
                                kt[p][e * 64:(e + 1) * 64,
                                      jc * 128:(jc + 1) * 128],
                                qt[p][e * 64:(e + 1) * 64, i0 + a:i0 + 512],
                                start=True, stop=True,
                                skip_group_check=True,
                            )
                        # AV of the previously-held group keeps PE fed while
                        # ACT runs this group's exp; crosses head boundaries.
                        if held is not None:
                            emit_avs(held)
                            held = None
                        if g == 0 and tt >= 1 and 1 <= h <= 3:
                            # previous slab's outproj + RS; RS one head after
                            # the last y_part write so the collective's dep
                            # wait doesn't block Pool.SEQ (causal masks)
                            if h <= 2:
                                for g4 in range(4):
                                    emit_outproj_group(tt - 1, 4 * (h - 1) + g4)
                            else:
                                emit_rs(tt - 1)
                        if g == 1 and pending is not None:
                            emit_norm(pending)
                            pending = None
                        if g == ngroups - 1 and tt < TTn - 1:
                            # next slab's projections as PE filler
                            emit_a_group(tt + 1, h)
                        pt = pp.tile([128, 1024], BF16, tag="pt", bufs=10,
                                     name=f"pt{tt}{h}{g}")
                        a0 = kk[0][1]
                        nc.scalar.activation(
                            pt[:, a0:1024], st[:, a0:1024],
                            mybir.ActivationFunctionType.Exp,
                        )
                        kb0 = 2 * g - 4 * tt
                        if kb0 >= 0:
                            # merged causal zeroing for diag pair (kb0, kb0+1):
                            # iota = i - 128*o - p over block-local cols
                            off = kb0 * 128
                            sel = pt[:].rearrange(
                                "p (o i) -> p o i", o=2, i=512
                            )[:, :, off:off + 256]
                            nc.gpsimd.affine_select(
                                out=sel, in_=sel,
                                compare_op=mybir.AluOpType.is_ge,
                                fill=0.0, base=0,
                                pattern=[[-128, 2], [1, 256]],
                                channel_multiplier=-1,
                            )
                        pts.append(pt)
                    held = (ov, h, tt, pts)
                    if pending is not None:
                        emit_norm(pending)
                        pending = None
                    pending = (ov, h, tt)
                if tt == TTn - 1:
                    if pending is not None:
                        emit_norm(pending)
                        pending = None
                    for cp in range(8):
                        emit_outproj_group(tt, cp)
                    emit_rs(tt)

    nc.compile()
    return nc


_NC_CACHE = {}


def _get_nc(with_rs: bool = True):
    key = bool(with_rs)
    if key not in _NC_CACHE:
        _NC_CACHE[key] = build_nc(with_rs)
    return _NC_CACHE[key]


def make_in_maps(x, Wq, Wk, Wv, Wo, bo):
    import ml_dtypes
    bf16 = ml_dtypes.bfloat16

    x = np.asarray(x, dtype=np.float32)
    Wq = np.asarray(Wq, dtype=np.float32)
    Wk = np.asarray(Wk, dtype=np.float32)
    Wv = np.asarray(Wv, dtype=np.float32)
    Wo = np.asarray(Wo, dtype=np.float32)
    bo = np.asarray(bo, dtype=np.float32)

    scale = np.float32(C) ** np.float32(-0.5)
    in_maps = []
    for c in range(N_CORES):
        b, hoff = c // 2, (c % 2) * HC
        heads = slice(hoff, hoff + HC)
        xT_c = np.ascontiguousarray(x[b].T.astype(bf16))              # [C, T]
        wq_c = np.ascontiguousarray(
            np.concatenate(list(Wq[heads] * scale), axis=1).astype(bf16))
        wk_c = np.ascontiguousarray(
            np.concatenate(list(Wk[heads]), axis=1).astype(bf16))
        wv_c = np.ascontiguousarray(
            np.concatenate(list(Wv[heads]), axis=1).astype(bf16))
        wot_c = np.ascontiguousarray(
            Wo[:, hoff * D:(hoff + HC) * D].T.astype(bf16))           # [512, C]
        bo2_c = np.ascontiguousarray((bo / 2.0).reshape(8, 128).T)    # [128, 8]
        in_maps.append({
            "xT": xT_c, "wq": wq_c, "wk": wk_c, "wv": wv_c,
            "wot": wot_c, "bo2": bo2_c,
        })
    return in_maps


def kernel(x, Wq, Wk, Wv, Wo, bo):
    nc = _get_nc(with_rs=True)
    in_maps = make_in_maps(x, Wq, Wk, Wv, Wo, bo)
    # The axon-tunneled devices occasionally fail transiently
    # (NRT_EXEC_UNIT_UNRECOVERABLE / tunnel hangup); a retry recovers.
    last_err = None
    for _ in range(3):
        try:
            res = run_bass_kernel_spmd(nc, in_maps, list(range(N_CORES))).results
            break
        except Exception as e:  # noqa: BLE001
            last_err = e
            import time
            time.sleep(5)
    else:
        raise last_err

    out = np.empty((B, T, C), dtype=np.float32)
    for c in range(N_CORES):
        b, e = c // 2, c % 2
        yc = np.asarray(res[c]["y"], dtype=np.float32)  # [tt, c' slab, t]
        for tt in range(TTn):
            out[b, tt * 512:(tt + 1) * 512, e * 512:(e + 1) * 512] = yc[tt].T
    return out


# revision 15
# speedup vs baseline: 1.2464x; 1.0084x over previous
"""Multi-head causal attention (B=4, T=2048, C=1024, H=16, D=64) on 8 trn2 cores.

Sharding: tensor-parallel over heads within batch core-pairs.
  core c -> batch b = c//2, heads hoff..hoff+7 where hoff = (c%2)*8.

Per-core pipeline (all matmul operands bf16, accumulation f32 in PSUM):
  - projections per t-slab (Q^T/K^T head-pair packed to 128 partitions; V
    head-packed with a ones column folded in for free softmax sums);
    slab tt+1's projections are interleaved into slab tt's attention so the
    PE fills the exp-latency gaps (attention is ACT-bound).
  - causal attention per head in S^T = [j, i] orientation, exp without
    max-subtraction (scores ~N(0, 0.25^2), safe); merged affine_select
    causal masks (one per diagonal chunk-pair).
  - AV in flipped orientation: stationary = P^T chunk [128 k, 128 q],
    moving = V [128 k, 65] -> O accumulates as [q, d|l] in PSUM; fully
    masked (q-chunk < key-chunk) matmuls skipped. Softmax normalization is
    then a per-partition tensor_scalar multiply; O^T rebuilt with PE
    transposes for the output projection.
  - output projection to partial y^T [1024 c', 2048 t] (+ bo/2) in bf16
  - pairwise ReduceScatter (bf16, per t-slab) sums partner partials; core
    even keeps c' 0:512, odd keeps c' 512:1024.
Host reassembles the [B, T, C] f32 output by transposing/concatenating.
"""

import numpy as np

import concourse.bass as bass
import concourse.mybir as mybir
from concourse import bacc, masks
from concourse.tile import TileContext
from concourse.bass_utils import run_bass_kernel_spmd

F32 = mybir.dt.float32
BF16 = mybir.dt.bfloat16

B, T, C = 4, 2048, 1024
H, D = 16, 64
HC = 8           # heads per core
NPAIR = HC // 2  # head pairs (QK packing)
CCn = C // 128   # 8 contraction chunks
TTn = T // 512   # 4 query slabs of 512
JCn = T // 128   # 16 key chunks of 128
N_CORES = 8
RG = [[0, 1], [2, 3], [4, 5], [6, 7]]


def build_nc(with_rs: bool = True):
    nc = bacc.Bacc(None, target_bir_lowering=False)

    xT = nc.declare_dram_parameter("xT", [C, T], BF16, isOutput=False)
    wq = nc.declare_dram_parameter("wq", [C, 512], BF16, isOutput=False)
    wk = nc.declare_dram_parameter("wk", [C, 512], BF16, isOutput=False)
    wv = nc.declare_dram_parameter("wv", [C, 512], BF16, isOutput=False)
    wot = nc.declare_dram_parameter("wot", [512, C], BF16, isOutput=False)
    bo2 = nc.declare_dram_parameter("bo2", [128, 8], F32, isOutput=False)
    y = nc.declare_dram_parameter("y", [TTn, 512, 512], BF16, isOutput=True)

    with TileContext(nc) as tc:
        with (
            tc.tile_pool(name="persist", bufs=1) as pp,
            tc.tile_pool(name="psum", bufs=1, space="PSUM") as psum,
            tc.tile_pool(name="dram", bufs=1, space="DRAM") as dram,
        ):
            # ---- persistent tiles ----
            qt = [pp.tile([128, T], BF16, tag=f"qt{p}", name=f"qt{p}")
                  for p in range(NPAIR)]
            kt = [pp.tile([128, T], BF16, tag=f"kt{p}", name=f"kt{p}")
                  for p in range(NPAIR)]
            # V chunks: 8 heads * 65 cols (64 d + ones col for softmax sums)
            v = [pp.tile([128, 65 * HC], BF16, tag=f"v{j}", name=f"v{j}")
                 for j in range(JCn)]
            ot = [pp.tile([128, T], BF16, tag=f"ot{p}", name=f"ot{p}")
                  for p in range(NPAIR)]
            wqt = [pp.tile([128, 512], BF16, tag=f"wq{cc}", name=f"wq{cc}")
                   for cc in range(CCn)]
            wkt = [pp.tile([128, 512], BF16, tag=f"wk{cc}", name=f"wk{cc}")
                   for cc in range(CCn)]
            wvt = [pp.tile([128, 512], BF16, tag=f"wv{cc}", name=f"wv{cc}")
                   for cc in range(CCn)]
            wot_t = [pp.tile([128, C], BF16, tag=f"wot{cl}", name=f"wot{cl}")
                     for cl in range(4)]
            ones8 = pp.tile([128, HC], BF16, tag="ones8")
            nc.vector.memset(ones8[:], 1.0)
            ident = pp.tile([128, 128], BF16, tag="ident")
            masks.make_identity(nc, ident[:])
            bo_sb = pp.tile([128, 8], F32, tag="bo_sb")

            y_part = dram.tile([TTn, 1024, 512], BF16)
            rs_out = dram.tile([TTn, 512, 512], BF16)

            # ---- startup DMAs: wq/x0 interleaved so Q proj starts early ----
            xts_store = {}

            def issue_xts(tt, interleave_with=None):
                i0 = tt * 512
                xts = [pp.tile([128, 512], BF16, tag=f"xt{cc}", bufs=2,
                               name=f"xt{cc}_{tt}") for cc in range(CCn)]
                for cc in range(CCn):
                    if interleave_with is not None:
                        nc.sync.dma_start(out=interleave_with[cc][:],
                                          in_=wq[cc * 128:(cc + 1) * 128, :])
                    nc.sync.dma_start(
                        out=xts[cc][:], in_=xT[cc * 128:(cc + 1) * 128, i0:i0 + 512]
                    )
                xts_store[tt] = xts

            issue_xts(0, interleave_with=wqt)
            for cc in range(CCn):
                nc.sync.dma_start(out=wkt[cc][:], in_=wk[cc * 128:(cc + 1) * 128, :])
            for cc in range(CCn):
                nc.sync.dma_start(out=wvt[cc][:], in_=wv[cc * 128:(cc + 1) * 128, :])
            for cl in range(4):
                nc.sync.dma_start(out=wot_t[cl][:],
                                  in_=wot[cl * 128:(cl + 1) * 128, :])
            nc.sync.dma_start(out=bo_sb[:], in_=bo2[:])

            # ---- projection emission (phase A), one group at a time ----
            def emit_a_group(tt, gi):
                """gi 0-1: Q pair-halves; 2-3: K; 4-7: V chunks."""
                i0 = tt * 512
                xts = xts_store[tt]
                if gi < 4:
                    # yps tag (not st): keeps this filler work off the QK
                    # score-psum rotation so it can't stall the exp pipeline
                    wt, dst = (wqt, qt) if gi < 2 else (wkt, kt)
                    p2 = gi % 2
                    for k in range(2):
                        p = 2 * p2 + k
                        ps = psum.tile([128, 512], F32, tag="yps", bufs=2,
                                       name=f"aps{tt}_{gi}_{k}")
                        for cc in range(CCn):
                            nc.tensor.matmul(
                                ps[:],
                                wt[cc][:, p * 128:(p + 1) * 128],
                                xts[cc][:],
                                start=(cc == 0), stop=(cc == CCn - 1),
                                skip_group_check=True,
                            )
                        nc.vector.tensor_copy(dst[p][:, i0:i0 + 512], ps[:])
                else:
                    jc = 4 * tt + (gi - 4)
                    jl = jc * 128 - i0
                    ps = psum.tile([128, 512], F32, tag="yps", bufs=2,
                                   name=f"vps{jc}")
                    for g in range(2):
                        for cc in range(CCn):
                            nc.tensor.matmul(
                                ps[:, g * 256:(g + 1) * 256],
                                xts[cc][:, jl:jl + 128],
                                wvt[cc][:, g * 256:(g + 1) * 256],
                                start=(cc == 0), stop=(cc == CCn - 1),
                                skip_group_check=True,
                            )
                    vre = v[jc][:].rearrange("p (h e) -> p h e", h=HC, e=65)
                    nc.vector.tensor_copy(vre[:, :, 0:64], ps[:])
                    nc.vector.tensor_copy(vre[:, :, 64:65], ones8[:])

            # ---- attention phase ----
            held = None     # AV batch awaiting emission
            pending = None  # (ov, h, tt) awaiting normalization

            def emit_avs(h_, upto=4):
                # whole-head AV, qc-major: PSUM allows only one open
                # accumulation group per bank, so each qc region's chunk
                # accumulation must be contiguous. Split qc 0-1 / 2-3 so the
                # part depending on the head's last causal mask goes out a
                # group later (mask latency hidden by the next head's QK).
                ov, h, tt, pts = h_[0], h_[1], h_[2], h_[3]
                for qc in range(h_[4], upto):
                    for jc in range(4 * tt + qc + 1):
                        g2, k = jc // 2, jc % 2
                        nc.tensor.matmul(
                            ov[:, qc * 65:(qc + 1) * 65],
                            pts[g2][:, k * 512 + qc * 128:k * 512 + (qc + 1) * 128],
                            v[jc][:, h * 65:(h + 1) * 65],
                            start=(jc == 0), stop=(jc == 4 * tt + qc),
                            skip_group_check=True,
                        )
                h_[4] = upto

            def emit_norm(pend):
                nonlocal held
                ov, h, tt = pend
                if held is not None and held[0] is ov:
                    emit_avs(held)
                    held = None
                p, e, i0 = h // 2, h % 2, tt * 512
                ovv = ov[:].rearrange("p (q f) -> p q f", q=4, f=65)
                rl4 = pp.tile([128, 4], F32, tag="rl4", bufs=2)
                nc.vector.reciprocal(rl4[:], ovv[:, :, 64:65])
                o_sb = pp.tile([128, 256], BF16, tag="osb", bufs=2)
                for qc in range(4):
                    nc.vector.tensor_scalar_mul(
                        o_sb[:, qc * 64:(qc + 1) * 64],
                        ov[:, qc * 65:qc * 65 + 64],
                        rl4[:, qc:qc + 1],
                    )
                tps = psum.tile([64, 512], BF16, tag="yps", bufs=2,
                                name=f"tps{tt}{h}")
                for qc in range(4):
                    nc.tensor.matmul(
                        tps[:, qc * 128:(qc + 1) * 128],
                        o_sb[:, qc * 64:(qc + 1) * 64],
                        ident[:],
                        is_transpose=True, start=True, stop=True,
                        skip_group_check=True,
                    )
                nc.vector.tensor_copy(ot[p][e * 64:(e + 1) * 64, i0:i0 + 512],
                                      tps[:])

            def emit_outproj_group(tt, cp):
                i0 = tt * 512
                yps = psum.tile([128, 512], F32, tag="yps", bufs=2,
                                name=f"yps{tt}{cp}")
                for cl in range(4):
                    nc.tensor.matmul(
                        yps[:],
                        wot_t[cl][:, cp * 128:(cp + 1) * 128],
                        ot[cl][:, i0:i0 + 512],
                        start=(cl == 0), stop=(cl == 3),
                        skip_group_check=True,
                    )
                ysb = pp.tile([128, 512], BF16, tag="ysb", bufs=4)
                nc.vector.tensor_scalar_add(ysb[:], yps[:], bo_sb[:, cp:cp + 1])
                nc.sync.dma_start(
                    out=y_part[tt, cp * 128:(cp + 1) * 128, :], in_=ysb[:]
                )

            def emit_rs(tt):
                if with_rs:
                    nc.gpsimd.collective_compute(
                        "ReduceScatter",
                        mybir.AluOpType.add,
                        replica_groups=RG,
                        ins=[y_part[tt]],
                        outs=[rs_out[tt]],
                    )
                    nc.sync.dma_start(out=y[tt], in_=rs_out[tt])
                else:
                    nc.sync.dma_start(out=y[tt], in_=y_part[tt, 0:512, :])

            # standalone projections for slab 0
            for gi in range(8):
                emit_a_group(0, gi)

            for tt in range(TTn):
                i0 = tt * 512
                n_jc = 4 * (tt + 1)
                ngroups = n_jc // 2
                if tt < TTn - 1:
                    issue_xts(tt + 1)
                for h in range(HC):
                    p, e = h // 2, h % 2
                    ov = psum.tile([128, 260], F32, tag="ov", bufs=2,
                                   name=f"ov{tt}{h}")
                    pts = []
                    for g in range(ngroups):
                        st = psum.tile([128, 1024], F32, tag="st", bufs=2,
                                       name=f"st{tt}{h}{g}")
                        kk = []
                        for k in range(2):
                            jc = 2 * g + k
                            kb = jc - 4 * tt
                            cap = 256 if k == 0 else 384
                            a = min(kb * 128, cap) if kb >= 0 else 0
                            kk.append((jc, a))
                            nc.tensor.matmul(
                                st[:, k * 512 + a:(k + 1) * 512],
                                kt[p][e * 64:(e + 1) * 64,
                                      jc * 128:(jc + 1) * 128],
                                qt[p][e * 64:(e + 1) * 64, i0 + a:i0 + 512],
                                start=True, stop=True,
                                skip_group_check=True,
                            )
                        # AV of the previously-held head keeps PE fed while
                        # ACT runs this group's exp; crosses head boundaries.
                        if held is not None:
                            if g == 0:
                                emit_avs(held, upto=2)
                            else:
                                emit_avs(held, upto=4)
                                held = None
                        if g == 0 and tt >= 1 and 1 <= h <= 3:
                            # previous slab's outproj + RS; RS one head after
                            # the last y_part write so the collective's dep
                            # wait doesn't block Pool.SEQ (causal masks)
                            if h <= 2:
                                for g4 in range(4):
                                    emit_outproj_group(tt - 1, 4 * (h - 1) + g4)
                            else:
                                emit_rs(tt - 1)
                        if g == 1 and pending is not None:
                            emit_norm(pending)
                            pending = None
                        if g == ngroups - 1 and tt < TTn - 1:
                            # next slab's projections as PE filler
                            emit_a_group(tt + 1, h)
                        pt = pp.tile([128, 1024], BF16, tag="pt", bufs=10,
                                     name=f"pt{tt}{h}{g}")
                        a0 = kk[0][1]
                        nc.scalar.activation(
                            pt[:, a0:1024], st[:, a0:1024],
                            mybir.ActivationFunctionType.Exp,
                        )
                        kb0 = 2 * g - 4 * tt
                        if kb0 >= 0:
                            # merged causal zeroing for diag pair (kb0, kb0+1):
                            # iota = i - 128*o - p over block-local cols
                            off = kb0 * 128
                            sel = pt[:].rearrange(
                                "p (o i) -> p o i", o=2, i=512
                            )[:, :, off:off + 256]
                            nc.gpsimd.affine_select(
                                out=sel, in_=sel,
                                compare_op=mybir.AluOpType.is_ge,
                                fill=0.0, base=0,
                                pattern=[[-128, 2], [1, 256]],
                                channel_multiplier=-1,
                            )
                        pts.append(pt)
                    held = [ov, h, tt, pts, 0]
                    if pending is not None:
                        emit_norm(pending)
                        pending = None
                    pending = (ov, h, tt)
                if tt == TTn - 1:
                    if pending is not None:
                        emit_norm(pending)
                        pending = None
                    for cp in range(8):
                        emit_outproj_group(tt, cp)
                    emit_rs(tt)

    nc.compile()
    return nc


_NC_CACHE = {}


def _get_nc(with_rs: bool = True):
    key = bool(with_rs)
    if key not in _NC_CACHE:
        _NC_CACHE[key] = build_nc(with_rs)
    return _NC_CACHE[key]


def make_in_maps(x, Wq, Wk, Wv, Wo, bo):
    import ml_dtypes
    bf16 = ml_dtypes.bfloat16

    x = np.asarray(x, dtype=np.float32)
    Wq = np.asarray(Wq, dtype=np.float32)
    Wk = np.asarray(Wk, dtype=np.float32)
    Wv = np.asarray(Wv, dtype=np.float32)
    Wo = np.asarray(Wo, dtype=np.float32)
    bo = np.asarray(bo, dtype=np.float32)

    scale = np.float32(C) ** np.float32(-0.5)
    in_maps = []
    for c in range(N_CORES):
        b, hoff = c // 2, (c % 2) * HC
        heads = slice(hoff, hoff + HC)
        xT_c = np.ascontiguousarray(x[b].T.astype(bf16))              # [C, T]
        wq_c = np.ascontiguousarray(
            np.concatenate(list(Wq[heads] * scale), axis=1).astype(bf16))
        wk_c = np.ascontiguousarray(
            np.concatenate(list(Wk[heads]), axis=1).astype(bf16))
        wv_c = np.ascontiguousarray(
            np.concatenate(list(Wv[heads]), axis=1).astype(bf16))
        wot_c = np.ascontiguousarray(
            Wo[:, hoff * D:(hoff + HC) * D].T.astype(bf16))           # [512, C]
        bo2_c = np.ascontiguousarray((bo / 2.0).reshape(8, 128).T)    # [128, 8]
        in_maps.append({
            "xT": xT_c, "wq": wq_c, "wk": wk_c, "wv": wv_c,
            "wot": wot_c, "bo2": bo2_c,
        })
    return in_maps


def kernel(x, Wq, Wk, Wv, Wo, bo):
    nc = _get_nc(with_rs=True)
    in_maps = make_in_maps(x, Wq, Wk, Wv, Wo, bo)
    # The axon-tunneled devices occasionally fail transiently
    # (NRT_EXEC_UNIT_UNRECOVERABLE / tunnel hangup); a retry recovers.
    last_err = None
    for _ in range(3):
        try:
            res = run_bass_kernel_spmd(nc, in_maps, list(range(N_CORES))).results
            break
        except Exception as e:  # noqa: BLE001
            last_err = e
            import time
            time.sleep(5)
    else:
        raise last_err

    out = np.empty((B, T, C), dtype=np.float32)
    for c in range(N_CORES):
        b, e = c // 2, c % 2
        yc = np.asarray(res[c]["y"], dtype=np.float32)  # [tt, c' slab, t]
        for tt in range(TTn):
            out[b, tt * 512:(tt + 1) * 512, e * 512:(e + 1) * 512] = yc[tt].T
    return out
